# revision 5
# baseline (speedup 1.0000x reference)
"""Trainium2 Bass kernel for the GRU seq2seq AR model.

Model (reference): GRU encoder over S=1024 steps, then T=256 autoregressive
decoder steps (teacher_forcing_rate=0, so decoder input is always its own
previous output y = Wl @ h + bl).

Strategy:
  - Pure data parallel: batch 512 sharded 8 ways (64 rows/core), weights
    replicated, zero collectives.
  - Per step, ONE fused matmul group computes all gate pre-activations:
      g = lhsT.T @ [inp; 1; h]   (K = 64+1+512 -> 5 K-tiles of <=128)
    with M-columns laid out as [r|z|i_n|h_n|y] (512,512,512,512,64).
    A ones-row folds all biases into the matmul. r/z/h_n weight columns are
    pre-scaled by 0.5 so gates reduce to tanh only
    (sigmoid(a) = .5 + .5*tanh(.5a)), avoiding ACT table switches.
  - Decoder: Wl is FOLDED into the gate weights (inp = Wl@h + bl always), so
    the decoder has no serial y-feedback; the y chunk is output-only.
  - bf16 matmul inputs, fp32 PSUM + fp32 gate math + fp32 hidden master.

Layouts (per core, BC = 64):
  w    DRAM [10, 128, 2112] bf16: k-tiles 0:5 encoder, 5:10 decoder.
  xh   DRAM [65, S*BC] bf16: rows 0:64 = x[t].T steps, row 64 = ones.
  y    DRAM [64, (T+1)*BC] f32: slot d holds Wl@h^{(d)}+bl ([I, BC] each).
"""

import numpy as np
import ml_dtypes

B, S, I, H, T = 512, 1024, 64, 512, 256
NCORES = 8
BC = B // NCORES
BF16 = ml_dtypes.bfloat16

# M-column blocks inside each 2112-col weight tile
_R0, _Z0, _N0, _H0, _Y0 = 0, 512, 1024, 1536, 2048
WCOLS = 2112
# PSUM column blocks ([128, 1088] f32): r z i_n h_n | y
_PR, _PZ, _PN, _PH, _PY = 0, 256, 512, 768, 1024


def _build_weights(Wi, Wh, bi, bh, Wl, bl):
    """lhsT tiles [10, 128, 2112] fp32 -> bf16.
    K rows: k0 = [x(64); ones(1)], k1..k4 = h chunks of 128.
    Tiles 0:5 = encoder, 5:10 = decoder (Wl folded)."""
    w = np.zeros((10, 128, WCOLS), np.float32)

    def fill(base, Wx, bx_r, bx_z, bx_n, Whh, Win_, x_has_w):
        # k0: x rows (encoder only) + bias row
        if x_has_w:
            w[base, 0:64, _R0:_R0 + 512] = 0.5 * Wx.T[:, 0:512]
            w[base, 0:64, _Z0:_Z0 + 512] = 0.5 * Wx.T[:, 512:1024]
            w[base, 0:64, _N0:_N0 + 512] = Wx.T[:, 1024:1536]
        w[base, 64, _R0:_R0 + 512] = 0.5 * bx_r
        w[base, 64, _Z0:_Z0 + 512] = 0.5 * bx_z
        w[base, 64, _N0:_N0 + 512] = bx_n
        w[base, 64, _H0:_H0 + 512] = 0.5 * bh[1024:1536]
        w[base, 64, _Y0:_Y0 + 64] = bl
        for c in range(4):
            hs = slice(128 * c, 128 * (c + 1))
            w[base + 1 + c, :, _R0:_R0 + 512] = 0.5 * Whh.T[hs, 0:512]
            w[base + 1 + c, :, _Z0:_Z0 + 512] = 0.5 * Whh.T[hs, 512:1024]
            if Win_ is not None:
                w[base + 1 + c, :, _N0:_N0 + 512] = Win_.T[hs, :]
            w[base + 1 + c, :, _H0:_H0 + 512] = 0.5 * Wh[1024:1536].T[hs, :]
            w[base + 1 + c, :, _Y0:_Y0 + 64] = Wl.T[hs, :]

    # encoder: gi from x via k0; gh from h
    fill(0, Wi, (bi + bh)[0:512], (bi + bh)[512:1024], bi[1024:1536],
         Wh[0:1024], None, x_has_w=True)
    # decoder: inp = Wl@h + bl folded -> all through h rows
    Wc = Wh[0:1024] + Wi[0:1024] @ Wl
    Win = Wi[1024:1536] @ Wl
    fill(5, Wi,
         (bi + bh)[0:512] + Wi[0:512] @ bl,
         (bi + bh)[512:1024] + Wi[512:1024] @ bl,
         bi[1024:1536] + Wi[1024:1536] @ bl,
         Wc, Win, x_has_w=False)
    return w.astype(BF16)


def _build_x(xc):
    """xc [BC, S, I] -> [65, S*BC] bf16 with ones row."""
    s = xc.shape[1]
    xt = np.ones((65, s, BC), np.float32)
    xt[0:64] = xc.transpose(2, 1, 0)
    return np.ascontiguousarray(xt.reshape(65, s * BC)).astype(BF16)


def build_program(s_steps=S, t_steps=T, ue=32, ud=32, use_loops=True):
    """Build the Bass/Tile program (shared by all 8 cores)."""
    from contextlib import ExitStack
    import concourse.bass as bass
    import concourse.bacc as bacc
    import concourse.mybir as mybir
    import concourse.tile as tile

    f32 = mybir.dt.float32
    bf16 = mybir.dt.bfloat16
    TANH = mybir.ActivationFunctionType.Tanh
    COPY = mybir.ActivationFunctionType.Copy
    MUL = mybir.AluOpType.mult
    ADD = mybir.AluOpType.add

    assert s_steps % ue == 0 and t_steps % ud == 0

    nc = bacc.Bacc("TRN2", target_bir_lowering=False, debug=False,
                   num_devices=NCORES)
    w_ext = nc.declare_dram_parameter("w", [10, 128, WCOLS], bf16, isOutput=False)
    x_ext = nc.declare_dram_parameter("xh", [65, s_steps * BC], bf16, isOutput=False)
    y_ext = nc.declare_dram_parameter("y", [64, (t_steps + 1) * BC], f32, isOutput=True)

    with ExitStack() as ctx:
        tc = ctx.enter_context(tile.TileContext(nc))
        state = ctx.enter_context(tc.tile_pool(name="state", bufs=1))
        wpool = ctx.enter_context(tc.tile_pool(name="wpool", bufs=1))
        xpool = ctx.enter_context(tc.tile_pool(name="xpool", bufs=1))
        ypool = ctx.enter_context(tc.tile_pool(name="ypool", bufs=1))
        gp = ctx.enter_context(tc.tile_pool(name="gates", bufs=2))
        psum = ctx.enter_context(tc.tile_pool(name="psum", bufs=2, space="PSUM"))

        wte, wtd = [], []
        for k in range(10):
            t_ = wpool.tile([128, WCOLS], bf16, tag=f"w{k}")
            nc.sync.dma_start(t_[:], w_ext[k, :, :])
            (wte if k < 5 else wtd).append(t_)

        hbf = state.tile([128, 256], bf16, tag="hbf")    # h.T chunks (bf16)
        hf32 = state.tile([128, 256], f32, tag="hf32")   # h.T master (f32)
        rhs0d = state.tile([65, BC], bf16, tag="rhs0d")  # decoder k0 = [0...; 1]
        nc.vector.memset(hbf[:], 0.0)
        nc.vector.memset(hf32[:], 0.0)
        nc.vector.memset(rhs0d[:], 0.0)
        nc.vector.memset(rhs0d[64:65, :], 1.0)

        def emit_mms(g, rhs0, wt, enc, want_y):
            """g psum [128,1088]. cols: r 0:256 | z 256:512 | i_n 512:768 |
            h_n 768:1024 | y 1024:1088."""
            # PSUM rule: start=True clears has_written for the WHOLE bank, so
            # emit exactly one start (bank's first MM) and one stop (bank's
            # last MM) per step. flags=0 overwrites where has_written is
            # unset, accumulates where set — per element.
            # banks: b0 = r+z (cols 0:512), b1 = i_n+h_n (512:1024), b2 = y.
            hk = lambda k: hbf[:, (k - 1) * 64:k * 64]
            k0 = wt[0][0:65, :]
            # phase A: all k0 matmuls (bias row; x rows for encoder)
            for m in range(8):  # r, z
                nc.tensor.matmul(g[:, 64 * m:64 * m + 64],
                                 k0[:, 128 * m:128 * m + 128],
                                 rhs0, start=(m == 0), stop=False)
            for m in range(4):  # i_n
                nc.tensor.matmul(g[:, _PN + 64 * m:_PN + 64 * m + 64],
                                 k0[:, _N0 + 128 * m:_N0 + 128 * m + 128],
                                 rhs0, start=(m == 0), stop=False)
            for m in range(4):  # h_n (bias only in k0)
                nc.tensor.matmul(g[:, _PH + 64 * m:_PH + 64 * m + 64],
                                 k0[:, _H0 + 128 * m:_H0 + 128 * m + 128],
                                 rhs0, start=False, stop=False)
            if want_y:
                nc.tensor.matmul(g[0:64, _PY:_PY + 64],
                                 k0[:, _Y0:_Y0 + 64],
                                 rhs0, start=True, stop=False)
            # phase B: r,z over h k-tiles
            for m in range(8):
                for k in range(1, 5):
                    nc.tensor.matmul(g[:, 64 * m:64 * m + 64],
                                     wt[k][:, 128 * m:128 * m + 128],
                                     hk(k), start=False,
                                     stop=(m == 7 and k == 4))
            # phase C: i_n (decoder only) + h_n over h k-tiles
            if not enc:
                for m in range(4):
                    for k in range(1, 5):
                        nc.tensor.matmul(g[:, _PN + 64 * m:_PN + 64 * m + 64],
                                         wt[k][:, _N0 + 128 * m:_N0 + 128 * m + 128],
                                         hk(k), start=False, stop=False)
            for m in range(4):
                for k in range(1, 5):
                    nc.tensor.matmul(g[:, _PH + 64 * m:_PH + 64 * m + 64],
                                     wt[k][:, _H0 + 128 * m:_H0 + 128 * m + 128],
                                     hk(k), start=False,
                                     stop=(m == 3 and k == 4))
            # phase D: y over h k-tiles
            if want_y:
                for k in range(1, 5):
                    nc.tensor.matmul(g[0:64, _PY:_PY + 64],
                                     wt[k][:, _Y0:_Y0 + 64],
                                     hk(k), start=False, stop=(k == 4))

        def emit_gates(g, ytile=None, yslot=0):
            """Gate math; updates hbf/hf32 (and ytile when decoding)."""
            trz = gp.tile([128, 512], f32, tag="trz")
            zz2 = gp.tile([128, 256], f32, tag="zz2")
            nz2 = gp.tile([128, 256], f32, tag="nz2")
            q = gp.tile([128, 256], f32, tag="q")
            u = gp.tile([128, 256], f32, tag="u")
            v = gp.tile([128, 256], f32, tag="v")
            an = gp.tile([128, 256], f32, tag="an")
            tn = gp.tile([128, 256], f32, tag="tn")
            mm1 = gp.tile([128, 256], f32, tag="mm1")

            tr = trz[:, 0:256]
            tz = trz[:, 256:512]
            g_in = g[:, _PN:_PN + 256]
            g_hn = g[:, _PH:_PH + 256]

            nc.scalar.activation(trz[:], g[:, 0:512], TANH)
            if ytile is not None:
                nc.scalar.activation(
                    ytile[:, yslot * BC:(yslot + 1) * BC], g[0:64, _PY:_PY + 64],
                    COPY)
            nc.vector.tensor_scalar(zz2[:], tz, 0.5, 0.5, MUL, ADD)    # z
            nc.vector.tensor_scalar(nz2[:], tz, -0.5, 0.5, MUL, ADD)   # 1-z
            nc.vector.tensor_tensor(q[:], zz2[:], hf32[:], MUL)        # z*h
            nc.vector.tensor_tensor(u[:], tr, g_hn, MUL)               # tr*hn2
            nc.vector.tensor_tensor(v[:], u[:], g_in, ADD)             # +i_n
            nc.vector.tensor_tensor(an[:], v[:], g_hn, ADD)            # +hn2
            nc.scalar.activation(tn[:], an[:], TANH)                   # n
            nc.vector.tensor_tensor(mm1[:], nz2[:], tn[:], MUL)        # (1-z)*n
            nc.vector.tensor_tensor(hbf[:], mm1[:], q[:], ADD)         # h' bf16
            nc.vector.tensor_tensor(hf32[:], mm1[:], q[:], ADD)        # h' f32

        def enc_step(rhs0):
            g = psum.tile([128, 1088], f32, tag="g")
            emit_mms(g, rhs0, wte, enc=True, want_y=False)
            emit_gates(g)

        def dec_step(ytile, yslot):
            g = psum.tile([128, 1088], f32, tag="g")
            emit_mms(g, rhs0d[0:65, :], wtd, enc=False, want_y=True)
            emit_gates(g, ytile=ytile, yslot=yslot)

        PE = mybir.EngineType.PE
        DVE = mybir.EngineType.DVE

        # ---- encoder ----
        if use_loops:
            with tc.For_i(0, s_steps * BC, ue * BC, hint_engines=(PE, DVE)) as iv:
                xch = xpool.tile([65, ue * BC], bf16, tag="xch")
                nc.sync.dma_start(xch[:], x_ext[:, bass.ds(iv, ue * BC)])
                for j in range(ue):
                    enc_step(xch[:, j * BC:(j + 1) * BC])
        else:
            for i0 in range(0, s_steps, ue):
                xch = xpool.tile([65, ue * BC], bf16, tag="xch")
                nc.sync.dma_start(xch[:], x_ext[:, i0 * BC:(i0 + ue) * BC])
                for j in range(ue):
                    enc_step(xch[:, j * BC:(j + 1) * BC])

        # ---- decoder (no bridge needed: Wl folded, no y feedback) ----
        if use_loops:
            with tc.For_i(0, t_steps * BC, ud * BC, hint_engines=(PE, DVE)) as iv:
                yt = ypool.tile([64, ud * BC], f32, tag="yt")
                for j in range(ud):
                    dec_step(yt, j)
                nc.sync.dma_start(y_ext[:, bass.ds(iv, ud * BC)], yt[:])
        else:
            for d0 in range(0, t_steps, ud):
                yt = ypool.tile([64, ud * BC], f32, tag="yt")
                for j in range(ud):
                    dec_step(yt, j)
                nc.sync.dma_start(y_ext[:, d0 * BC:(d0 + ud) * BC], yt[:])

        # ---- tail: y for the final hidden state -> slot T ----
        gt_ = psum.tile([128, 1088], f32, tag="g")
        nc.tensor.matmul(gt_[0:64, _PY:_PY + 64], wtd[0][0:65, _Y0:_Y0 + 64],
                         rhs0d[0:65, :], start=True, stop=False)
        for k in range(1, 5):
            nc.tensor.matmul(gt_[0:64, _PY:_PY + 64], wtd[k][:, _Y0:_Y0 + 64],
                             hbf[:, (k - 1) * 64:k * 64], start=False, stop=(k == 4))
        ylast = ypool.tile([64, BC], f32, tag="ylast")
        nc.scalar.activation(ylast[:], gt_[0:64, _PY:_PY + 64], COPY)
        nc.sync.dma_start(y_ext[:, t_steps * BC:(t_steps + 1) * BC], ylast[:])

    nc.compile()
    return nc


def run(nc, w_bf, x_cores, trace=False):
    """Execute on 8 cores; returns per-core y arrays and BassKernelResults."""
    from concourse.bass_utils import run_bass_kernel_spmd
    in_maps = [{"w": w_bf, "xh": x_cores[c]} for c in range(NCORES)]
    res = run_bass_kernel_spmd(nc, in_maps, core_ids=list(range(NCORES)),
                               trace=trace)
    return [res.results[c]["y"] for c in range(NCORES)], res


_NC_CACHE = {}


def kernel(x, Wi, Wh, bi, bh, Wl, bl, targets=None, target_seq_len=T,
           teacher_forcing_rate=0, **_unused):
    x = np.asarray(x, np.float32)
    assert x.shape == (B, S, I), x.shape
    assert int(target_seq_len) == T
    w_bf = _build_weights(np.asarray(Wi, np.float32), np.asarray(Wh, np.float32),
                          np.asarray(bi, np.float32), np.asarray(bh, np.float32),
                          np.asarray(Wl, np.float32), np.asarray(bl, np.float32))
    x_cores = [_build_x(x[c * BC:(c + 1) * BC]) for c in range(NCORES)]

    key = (S, T)
    if key not in _NC_CACHE:
        _NC_CACHE[key] = build_program(S, T)
    ys, _ = run(_NC_CACHE[key], w_bf, x_cores)

    out = np.empty((B, T, I), np.float32)
    for c in range(NCORES):
        yc = ys[c].reshape(64, T + 1, BC)[:, 1:, :]   # [I, T, BC]
        out[c * BC:(c + 1) * BC] = yc.transpose(2, 1, 0)
    return out


if __name__ == "__main__":
    import reference
    inputs = reference.setup_inputs()
    out = kernel(**{k: np.asarray(v) if hasattr(v, "shape") else v
                    for k, v in inputs.items()})
    print("kernel out", out.shape, out.dtype)


# revision 8
# speedup vs baseline: 1.3548x; 1.3548x over previous
"""Trainium2 Bass kernel for the GRU seq2seq AR model.

Model (reference): GRU encoder over S=1024 steps, then T=256 autoregressive
decoder steps (teacher_forcing_rate=0, so decoder input is always its own
previous output y = Wl @ h + bl).

Strategy:
  - Pure data parallel: batch 512 sharded 8 ways (64 rows/core), weights
    replicated, zero collectives.
  - Per step, ONE fused matmul group computes all gate pre-activations:
      g = lhsT.T @ [inp; 1; h]   (K = 64+1+512 -> 5 K-tiles of <=128)
    with M-columns laid out as [r|z|i_n|h_n|y] (512,512,512,512,64).
    A ones-row folds all biases into the matmul. r/z/h_n weight columns are
    pre-scaled by 0.5 so gates reduce to tanh only
    (sigmoid(a) = .5 + .5*tanh(.5a)), avoiding ACT table switches.
  - Decoder: Wl is FOLDED into the gate weights (inp = Wl@h + bl always), so
    the decoder has no serial y-feedback; the y chunk is output-only.
  - bf16 matmul inputs, fp32 PSUM + fp32 gate math + fp32 hidden master.

Layouts (per core, BC = 64):
  w    DRAM [10, 128, 2112] bf16: k-tiles 0:5 encoder, 5:10 decoder.
  xh   DRAM [65, S*BC] bf16: rows 0:64 = x[t].T steps, row 64 = ones.
  y    DRAM [64, (T+1)*BC] f32: slot d holds Wl@h^{(d)}+bl ([I, BC] each).
"""

import numpy as np
import ml_dtypes

B, S, I, H, T = 512, 1024, 64, 512, 256
NCORES = 8
BC = B // NCORES
BF16 = ml_dtypes.bfloat16

# M-column blocks inside each 2112-col weight tile
_R0, _Z0, _N0, _H0, _Y0 = 0, 512, 1024, 1536, 2048
WCOLS = 2112


def _build_weights(Wi, Wh, bi, bh, Wl, bl):
    """lhsT tiles [10, 128, 2112] fp32 -> bf16.
    K rows: k0 = [x(64); ones(1)], k1..k4 = h chunks of 128.
    Tiles 0:5 = encoder, 5:10 = decoder (Wl folded)."""
    w = np.zeros((10, 128, WCOLS), np.float32)

    def fill(base, Wx, bx_r, bx_z, bx_n, Whh, Win_, x_has_w):
        # k0: x rows (encoder only) + bias row
        if x_has_w:
            w[base, 0:64, _R0:_R0 + 512] = 0.5 * Wx.T[:, 0:512]
            w[base, 0:64, _Z0:_Z0 + 512] = 0.5 * Wx.T[:, 512:1024]
            w[base, 0:64, _N0:_N0 + 512] = Wx.T[:, 1024:1536]
        w[base, 64, _R0:_R0 + 512] = 0.5 * bx_r
        w[base, 64, _Z0:_Z0 + 512] = 0.5 * bx_z
        w[base, 64, _N0:_N0 + 512] = bx_n
        w[base, 64, _H0:_H0 + 512] = 0.5 * bh[1024:1536]
        w[base, 64, _Y0:_Y0 + 64] = bl
        for c in range(4):
            hs = slice(128 * c, 128 * (c + 1))
            w[base + 1 + c, :, _R0:_R0 + 512] = 0.5 * Whh.T[hs, 0:512]
            w[base + 1 + c, :, _Z0:_Z0 + 512] = 0.5 * Whh.T[hs, 512:1024]
            if Win_ is not None:
                w[base + 1 + c, :, _N0:_N0 + 512] = Win_.T[hs, :]
            w[base + 1 + c, :, _H0:_H0 + 512] = 0.5 * Wh[1024:1536].T[hs, :]
            w[base + 1 + c, :, _Y0:_Y0 + 64] = Wl.T[hs, :]

    # encoder: gi from x via k0; gh from h
    fill(0, Wi, (bi + bh)[0:512], (bi + bh)[512:1024], bi[1024:1536],
         Wh[0:1024], None, x_has_w=True)
    # decoder: inp = Wl@h + bl folded -> all through h rows
    Wc = Wh[0:1024] + Wi[0:1024] @ Wl
    Win = Wi[1024:1536] @ Wl
    fill(5, Wi,
         (bi + bh)[0:512] + Wi[0:512] @ bl,
         (bi + bh)[512:1024] + Wi[512:1024] @ bl,
         bi[1024:1536] + Wi[1024:1536] @ bl,
         Wc, Win, x_has_w=False)
    return w.astype(BF16)


def _build_x(xc):
    """xc [BC, S, I] -> [65, S*BC] bf16 with ones row."""
    s = xc.shape[1]
    xt = np.ones((65, s, BC), np.float32)
    xt[0:64] = xc.transpose(2, 1, 0)
    return np.ascontiguousarray(xt.reshape(65, s * BC)).astype(BF16)


def build_program(s_steps=S, t_steps=T, ue=32, ud=32, use_loops=True):
    """Build the Bass/Tile program (shared by all 8 cores)."""
    from contextlib import ExitStack
    import concourse.bass as bass
    import concourse.bacc as bacc
    import concourse.mybir as mybir
    import concourse.tile as tile

    f32 = mybir.dt.float32
    bf16 = mybir.dt.bfloat16
    TANH = mybir.ActivationFunctionType.Tanh
    COPY = mybir.ActivationFunctionType.Copy
    MUL = mybir.AluOpType.mult
    ADD = mybir.AluOpType.add

    assert s_steps % ue == 0 and t_steps % ud == 0

    nc = bacc.Bacc("TRN2", target_bir_lowering=False, debug=False,
                   num_devices=NCORES)
    w_ext = nc.declare_dram_parameter("w", [10, 128, WCOLS], bf16, isOutput=False)
    x_ext = nc.declare_dram_parameter("xh", [65, s_steps * BC], bf16, isOutput=False)
    y_ext = nc.declare_dram_parameter("y", [64, (t_steps + 1) * BC], f32, isOutput=True)

    with ExitStack() as ctx:
        tc = ctx.enter_context(tile.TileContext(nc))
        state = ctx.enter_context(tc.tile_pool(name="state", bufs=1))
        wpool = ctx.enter_context(tc.tile_pool(name="wpool", bufs=1))
        xpool = ctx.enter_context(tc.tile_pool(name="xpool", bufs=1))
        ypool = ctx.enter_context(tc.tile_pool(name="ypool", bufs=1))
        gp = ctx.enter_context(tc.tile_pool(name="gates", bufs=2))
        psum = ctx.enter_context(tc.tile_pool(name="psum", bufs=2, space="PSUM"))

        wte, wtd = [], []
        for k in range(10):
            t_ = wpool.tile([128, WCOLS], bf16, tag=f"w{k}")
            nc.sync.dma_start(t_[:], w_ext[k, :, :])
            (wte if k < 5 else wtd).append(t_)

        hbf = state.tile([128, 256], bf16, tag="hbf")    # h.T chunks (bf16)
        hf32 = state.tile([128, 256], f32, tag="hf32")   # h.T master (f32)
        rhs0d = state.tile([65, BC], bf16, tag="rhs0d")  # decoder k0 = [0...; 1]
        nc.vector.memset(hbf[:], 0.0)
        nc.vector.memset(hf32[:], 0.0)
        nc.vector.memset(rhs0d[:], 0.0)
        nc.vector.memset(rhs0d[64:65, :], 1.0)

        def emit_mms(gr, gz, gnh, gy, rhs0, wt, enc, want_y):
            """Per-bank psum tiles: gr/gz [128,256] (r, z), gnh [128,512]
            (i_n | h_n), gy [64,64]. One start (first MM) and one stop (last
            MM) per psum tile per step — start=True clears the whole bank."""
            hk = lambda k: hbf[:, (k - 1) * 64:k * 64]
            k0 = wt[0][0:65, :]
            # phase A: all k0 matmuls (bias row; x rows for encoder)
            for m in range(4):  # r
                nc.tensor.matmul(gr[:, 64 * m:64 * m + 64],
                                 k0[:, _R0 + 128 * m:_R0 + 128 * m + 128],
                                 rhs0, start=(m == 0), stop=False)
            for m in range(4):  # z
                nc.tensor.matmul(gz[:, 64 * m:64 * m + 64],
                                 k0[:, _Z0 + 128 * m:_Z0 + 128 * m + 128],
                                 rhs0, start=(m == 0), stop=False)
            for m in range(4):  # i_n
                nc.tensor.matmul(gnh[:, 64 * m:64 * m + 64],
                                 k0[:, _N0 + 128 * m:_N0 + 128 * m + 128],
                                 rhs0, start=(m == 0), stop=False)
            for m in range(4):  # h_n (bias only in k0)
                nc.tensor.matmul(gnh[:, 256 + 64 * m:256 + 64 * m + 64],
                                 k0[:, _H0 + 128 * m:_H0 + 128 * m + 128],
                                 rhs0, start=False, stop=False)
            if want_y:
                nc.tensor.matmul(gy[:, :], k0[:, _Y0:_Y0 + 64],
                                 rhs0, start=True, stop=False)
            # phase B_r then B_z over h k-tiles (separate banks -> tanh(r)
            # can start while z still accumulates)
            for m in range(4):
                for k in range(1, 5):
                    nc.tensor.matmul(gr[:, 64 * m:64 * m + 64],
                                     wt[k][:, _R0 + 128 * m:_R0 + 128 * m + 128],
                                     hk(k), start=False,
                                     stop=(m == 3 and k == 4))
            for m in range(4):
                for k in range(1, 5):
                    nc.tensor.matmul(gz[:, 64 * m:64 * m + 64],
                                     wt[k][:, _Z0 + 128 * m:_Z0 + 128 * m + 128],
                                     hk(k), start=False,
                                     stop=(m == 3 and k == 4))
            # phase C: i_n (decoder only) + h_n over h k-tiles
            if not enc:
                for m in range(4):
                    for k in range(1, 5):
                        nc.tensor.matmul(gnh[:, 64 * m:64 * m + 64],
                                         wt[k][:, _N0 + 128 * m:_N0 + 128 * m + 128],
                                         hk(k), start=False, stop=False)
            for m in range(4):
                for k in range(1, 5):
                    nc.tensor.matmul(gnh[:, 256 + 64 * m:256 + 64 * m + 64],
                                     wt[k][:, _H0 + 128 * m:_H0 + 128 * m + 128],
                                     hk(k), start=False,
                                     stop=(m == 3 and k == 4))
            # phase D: y over h k-tiles
            if want_y:
                for k in range(1, 5):
                    nc.tensor.matmul(gy[:, :], wt[k][:, _Y0:_Y0 + 64],
                                     hk(k), start=False, stop=(k == 4))

        def emit_gates(gr, gz, gnh, gy, ytile=None, yslot=0):
            """Gate math; updates hbf/hf32 (and ytile when decoding).
            an = i_n + (1+tr)*hn2 keeps only 2 chained PSUM-source DVE ops."""
            tr_t = gp.tile([128, 256], f32, tag="tr_t")
            tz_t = gp.tile([128, 256], f32, tag="tz_t")
            tr1 = gp.tile([128, 256], f32, tag="tr1")
            zz2 = gp.tile([128, 256], f32, tag="zz2")
            nz2 = gp.tile([128, 256], f32, tag="nz2")
            q = gp.tile([128, 256], f32, tag="q")
            u2 = gp.tile([128, 256], f32, tag="u2")
            an = gp.tile([128, 256], f32, tag="an")
            tn = gp.tile([128, 256], f32, tag="tn")
            mm1 = gp.tile([128, 256], f32, tag="mm1")
            g_in = gnh[:, 0:256]
            g_hn = gnh[:, 256:512]

            # ACT: tanh only (no table switches)
            nc.scalar.activation(tr_t[:], gr[:, :], TANH)
            nc.scalar.activation(tz_t[:], gz[:, :], TANH)
            # DVE chain: tr1 -> u2 -> an -> (tn on ACT) -> mm1 -> hbf
            nc.vector.tensor_scalar(tr1[:], tr_t[:], 1.0, 0.0, ADD, ADD)
            nc.vector.tensor_tensor(u2[:], tr1[:], g_hn, MUL)     # (1+tr)*hn2
            nc.vector.tensor_tensor(an[:], u2[:], g_in, ADD)      # +i_n
            nc.scalar.activation(tn[:], an[:], TANH)              # n
            # prep ops inside the tn window
            nc.vector.tensor_scalar(zz2[:], tz_t[:], 0.5, 0.5, MUL, ADD)
            nc.vector.tensor_scalar(nz2[:], tz_t[:], -0.5, 0.5, MUL, ADD)
            if ytile is not None:
                nc.vector.tensor_copy(
                    ytile[:, yslot * BC:(yslot + 1) * BC], gy[:, :])
            nc.gpsimd.tensor_tensor(q[:], zz2[:], hf32[:], MUL)   # z*h
            nc.vector.tensor_tensor(mm1[:], nz2[:], tn[:], MUL)   # (1-z)*n
            nc.vector.tensor_tensor(hbf[:], mm1[:], q[:], ADD)    # h' bf16
            nc.vector.tensor_tensor(hf32[:], mm1[:], q[:], ADD)   # h' f32

        def alloc_psum():
            gr = psum.tile([128, 256], f32, tag="gr")
            gz = psum.tile([128, 256], f32, tag="gz")
            gnh = psum.tile([128, 512], f32, tag="gnh")
            gy = psum.tile([64, 64], f32, tag="gy")
            return gr, gz, gnh, gy

        def enc_step(rhs0):
            gr, gz, gnh, gy = alloc_psum()
            emit_mms(gr, gz, gnh, gy, rhs0, wte, enc=True, want_y=False)
            emit_gates(gr, gz, gnh, gy)

        def dec_step(ytile, yslot):
            gr, gz, gnh, gy = alloc_psum()
            emit_mms(gr, gz, gnh, gy, rhs0d[0:65, :], wtd, enc=False, want_y=True)
            emit_gates(gr, gz, gnh, gy, ytile=ytile, yslot=yslot)

        PE = mybir.EngineType.PE
        DVE = mybir.EngineType.DVE

        # ---- encoder ----
        if use_loops:
            with tc.For_i(0, s_steps * BC, ue * BC, hint_engines=(PE, DVE)) as iv:
                xch = xpool.tile([65, ue * BC], bf16, tag="xch")
                nc.sync.dma_start(xch[:], x_ext[:, bass.ds(iv, ue * BC)])
                for j in range(ue):
                    enc_step(xch[:, j * BC:(j + 1) * BC])
        else:
            for i0 in range(0, s_steps, ue):
                xch = xpool.tile([65, ue * BC], bf16, tag="xch")
                nc.sync.dma_start(xch[:], x_ext[:, i0 * BC:(i0 + ue) * BC])
                for j in range(ue):
                    enc_step(xch[:, j * BC:(j + 1) * BC])

        # ---- decoder (no bridge needed: Wl folded, no y feedback) ----
        if use_loops:
            with tc.For_i(0, t_steps * BC, ud * BC, hint_engines=(PE, DVE)) as iv:
                yt = ypool.tile([64, ud * BC], f32, tag="yt")
                for j in range(ud):
                    dec_step(yt, j)
                nc.sync.dma_start(y_ext[:, bass.ds(iv, ud * BC)], yt[:])
        else:
            for d0 in range(0, t_steps, ud):
                yt = ypool.tile([64, ud * BC], f32, tag="yt")
                for j in range(ud):
                    dec_step(yt, j)
                nc.sync.dma_start(y_ext[:, d0 * BC:(d0 + ud) * BC], yt[:])

        # ---- tail: y for the final hidden state -> slot T ----
        gy_t = psum.tile([64, 64], f32, tag="gy")
        nc.tensor.matmul(gy_t[:, :], wtd[0][0:65, _Y0:_Y0 + 64],
                         rhs0d[0:65, :], start=True, stop=False)
        for k in range(1, 5):
            nc.tensor.matmul(gy_t[:, :], wtd[k][:, _Y0:_Y0 + 64],
                             hbf[:, (k - 1) * 64:k * 64], start=False, stop=(k == 4))
        ylast = ypool.tile([64, BC], f32, tag="ylast")
        nc.vector.tensor_copy(ylast[:], gy_t[:, :])
        nc.sync.dma_start(y_ext[:, t_steps * BC:(t_steps + 1) * BC], ylast[:])

    nc.compile()
    return nc


def run(nc, w_bf, x_cores, trace=False):
    """Execute on 8 cores; returns per-core y arrays and BassKernelResults."""
    from concourse.bass_utils import run_bass_kernel_spmd
    in_maps = [{"w": w_bf, "xh": x_cores[c]} for c in range(NCORES)]
    res = run_bass_kernel_spmd(nc, in_maps, core_ids=list(range(NCORES)),
                               trace=trace)
    return [res.results[c]["y"] for c in range(NCORES)], res


_NC_CACHE = {}


def kernel(x, Wi, Wh, bi, bh, Wl, bl, targets=None, target_seq_len=T,
           teacher_forcing_rate=0, **_unused):
    x = np.asarray(x, np.float32)
    assert x.shape == (B, S, I), x.shape
    assert int(target_seq_len) == T
    w_bf = _build_weights(np.asarray(Wi, np.float32), np.asarray(Wh, np.float32),
                          np.asarray(bi, np.float32), np.asarray(bh, np.float32),
                          np.asarray(Wl, np.float32), np.asarray(bl, np.float32))
    x_cores = [_build_x(x[c * BC:(c + 1) * BC]) for c in range(NCORES)]

    key = (S, T)
    if key not in _NC_CACHE:
        _NC_CACHE[key] = build_program(S, T)
    ys, _ = run(_NC_CACHE[key], w_bf, x_cores)

    out = np.empty((B, T, I), np.float32)
    for c in range(NCORES):
        yc = ys[c].reshape(64, T + 1, BC)[:, 1:, :]   # [I, T, BC]
        out[c * BC:(c + 1) * BC] = yc.transpose(2, 1, 0)
    return out


if __name__ == "__main__":
    import reference
    inputs = reference.setup_inputs()
    out = kernel(**{k: np.asarray(v) if hasattr(v, "shape") else v
                    for k, v in inputs.items()})
    print("kernel out", out.shape, out.dtype)


# revision 10
# speedup vs baseline: 1.4928x; 1.1019x over previous
"""Trainium2 Bass kernel for the GRU seq2seq AR model.

Model (reference): GRU encoder over S=1024 steps, then T=256 autoregressive
decoder steps (teacher_forcing_rate=0, so decoder input is always its own
previous output y = Wl @ h + bl).

Strategy:
  - Pure data parallel: batch 512 sharded 8 ways (64 rows/core), weights
    replicated, zero collectives.
  - Per step, ONE fused matmul group computes all gate pre-activations:
      g = lhsT.T @ [inp; 1; h]   (K = 64+1+512 -> 5 K-tiles of <=128)
    with M-columns laid out as [r|z|i_n|h_n|y] (512,512,512,512,64).
    A ones-row folds all biases into the matmul. r/z/h_n weight columns are
    pre-scaled by 0.5 so gates reduce to tanh only
    (sigmoid(a) = .5 + .5*tanh(.5a)), avoiding ACT table switches.
  - Decoder: Wl is FOLDED into the gate weights (inp = Wl@h + bl always), so
    the decoder has no serial y-feedback; the y chunk is output-only.
  - bf16 matmul inputs, fp32 PSUM + fp32 gate math + fp32 hidden master.

Layouts (per core, BC = 64):
  w    DRAM [10, 128, 2112] bf16: k-tiles 0:5 encoder, 5:10 decoder.
  xh   DRAM [65, S*BC] bf16: rows 0:64 = x[t].T steps, row 64 = ones.
  y    DRAM [64, (T+1)*BC] f32: slot d holds Wl@h^{(d)}+bl ([I, BC] each).
"""

import numpy as np
import ml_dtypes

B, S, I, H, T = 512, 1024, 64, 512, 256
NCORES = 8
BC = B // NCORES
BF16 = ml_dtypes.bfloat16

# M-column blocks inside each 2112-col weight tile
_R0, _Z0, _N0, _H0, _Y0 = 0, 512, 1024, 1536, 2048
WCOLS = 2112


def _build_weights(Wi, Wh, bi, bh, Wl, bl):
    """lhsT tiles [10, 128, 2112] fp32 -> bf16.
    K rows: k0 = [x(64); ones(1)], k1..k4 = h chunks of 128.
    Tiles 0:5 = encoder, 5:10 = decoder (Wl folded)."""
    w = np.zeros((10, 128, WCOLS), np.float32)

    def fill(base, Wx, bx_r, bx_z, bx_n, Whh, Win_, x_has_w):
        # k0: x rows (encoder only) + bias row
        if x_has_w:
            w[base, 0:64, _R0:_R0 + 512] = 0.5 * Wx.T[:, 0:512]
            w[base, 0:64, _Z0:_Z0 + 512] = 0.5 * Wx.T[:, 512:1024]
            w[base, 0:64, _N0:_N0 + 512] = Wx.T[:, 1024:1536]
        w[base, 64, _R0:_R0 + 512] = 0.5 * bx_r
        w[base, 64, _Z0:_Z0 + 512] = 0.5 * bx_z
        w[base, 64, _N0:_N0 + 512] = bx_n
        w[base, 64, _H0:_H0 + 512] = 0.5 * bh[1024:1536]
        w[base, 64, _Y0:_Y0 + 64] = bl
        for c in range(4):
            hs = slice(128 * c, 128 * (c + 1))
            w[base + 1 + c, :, _R0:_R0 + 512] = 0.5 * Whh.T[hs, 0:512]
            w[base + 1 + c, :, _Z0:_Z0 + 512] = 0.5 * Whh.T[hs, 512:1024]
            if Win_ is not None:
                w[base + 1 + c, :, _N0:_N0 + 512] = Win_.T[hs, :]
            w[base + 1 + c, :, _H0:_H0 + 512] = 0.5 * Wh[1024:1536].T[hs, :]
            w[base + 1 + c, :, _Y0:_Y0 + 64] = Wl.T[hs, :]

    # encoder: gi from x via k0; gh from h
    fill(0, Wi, (bi + bh)[0:512], (bi + bh)[512:1024], bi[1024:1536],
         Wh[0:1024], None, x_has_w=True)
    # decoder: inp = Wl@h + bl folded -> all through h rows
    Wc = Wh[0:1024] + Wi[0:1024] @ Wl
    Win = Wi[1024:1536] @ Wl
    fill(5, Wi,
         (bi + bh)[0:512] + Wi[0:512] @ bl,
         (bi + bh)[512:1024] + Wi[512:1024] @ bl,
         bi[1024:1536] + Wi[1024:1536] @ bl,
         Wc, Win, x_has_w=False)
    return w.astype(BF16)


def _build_x(xc):
    """xc [BC, S, I] -> [65, S*BC] bf16 with ones row."""
    s = xc.shape[1]
    xt = np.ones((65, s, BC), np.float32)
    xt[0:64] = xc.transpose(2, 1, 0)
    return np.ascontiguousarray(xt.reshape(65, s * BC)).astype(BF16)


def build_program(s_steps=S, t_steps=T, ue=32, ud=32, use_loops=True):
    """Build the Bass/Tile program (shared by all 8 cores)."""
    from contextlib import ExitStack
    import concourse.bass as bass
    import concourse.bacc as bacc
    import concourse.mybir as mybir
    import concourse.tile as tile

    f32 = mybir.dt.float32
    bf16 = mybir.dt.bfloat16
    TANH = mybir.ActivationFunctionType.Tanh
    COPY = mybir.ActivationFunctionType.Copy
    MUL = mybir.AluOpType.mult
    ADD = mybir.AluOpType.add

    assert s_steps % ue == 0 and t_steps % ud == 0

    nc = bacc.Bacc("TRN2", target_bir_lowering=False, debug=False,
                   num_devices=NCORES)
    w_ext = nc.declare_dram_parameter("w", [10, 128, WCOLS], bf16, isOutput=False)
    x_ext = nc.declare_dram_parameter("xh", [65, s_steps * BC], bf16, isOutput=False)
    y_ext = nc.declare_dram_parameter("y", [64, (t_steps + 1) * BC], f32, isOutput=True)

    with ExitStack() as ctx:
        tc = ctx.enter_context(tile.TileContext(nc))
        state = ctx.enter_context(tc.tile_pool(name="state", bufs=1))
        wpool = ctx.enter_context(tc.tile_pool(name="wpool", bufs=1))
        xpool = ctx.enter_context(tc.tile_pool(name="xpool", bufs=1))
        ypool = ctx.enter_context(tc.tile_pool(name="ypool", bufs=1))
        gp = ctx.enter_context(tc.tile_pool(name="gates", bufs=2))
        psum = ctx.enter_context(tc.tile_pool(name="psum", bufs=2, space="PSUM"))

        wte, wtd = [], []
        for k in range(10):
            t_ = wpool.tile([128, WCOLS], bf16, tag=f"w{k}")
            nc.sync.dma_start(t_[:], w_ext[k, :, :])
            (wte if k < 5 else wtd).append(t_)

        hbf = state.tile([128, 256], bf16, tag="hbf")    # h.T chunks (bf16)
        hf32 = state.tile([128, 256], f32, tag="hf32")   # h.T master (f32)
        rhs0d = state.tile([65, BC], bf16, tag="rhs0d")  # decoder k0 = [0...; 1]
        nc.vector.memset(hbf[:], 0.0)
        nc.vector.memset(hf32[:], 0.0)
        nc.vector.memset(rhs0d[:], 0.0)
        nc.vector.memset(rhs0d[64:65, :], 1.0)

        def emit_mms(grz, gn, gh, gy, rhs0, wt, enc, want_y):
            """Per-bank psum tiles: grz [128,512] (r|z), gn [128,256] (i_n),
            gh [128,256] (h_n), gy [64,64]. One start (first MM) and one stop
            (last MM) per psum tile per step — start clears the whole bank."""
            hk = lambda k: hbf[:, (k - 1) * 64:k * 64]
            k0 = wt[0][0:65, :]
            # phase A: all k0 matmuls (bias row; x rows for encoder)
            for m in range(8):  # r, z
                nc.tensor.matmul(grz[:, 64 * m:64 * m + 64],
                                 k0[:, 128 * m:128 * m + 128],
                                 rhs0, start=(m == 0), stop=False)
            for m in range(4):  # i_n
                nc.tensor.matmul(gn[:, 64 * m:64 * m + 64],
                                 k0[:, _N0 + 128 * m:_N0 + 128 * m + 128],
                                 rhs0, start=(m == 0),
                                 stop=(enc and m == 3))
            for m in range(4):  # h_n (bias only in k0)
                nc.tensor.matmul(gh[:, 64 * m:64 * m + 64],
                                 k0[:, _H0 + 128 * m:_H0 + 128 * m + 128],
                                 rhs0, start=(m == 0), stop=False)
            if want_y:
                nc.tensor.matmul(gy[:, :], k0[:, _Y0:_Y0 + 64],
                                 rhs0, start=True, stop=False)
            # phase B: r,z — k-major so next step's k1/k2 can start on hbf
            # half-0 while half-1 gates still run
            for k in range(1, 5):
                for m in range(8):
                    nc.tensor.matmul(grz[:, 64 * m:64 * m + 64],
                                     wt[k][:, 128 * m:128 * m + 128],
                                     hk(k), start=False,
                                     stop=(m == 7 and k == 4))
            # phase C_h: h_n (chain head input) then C_n: i_n (decoder only)
            for k in range(1, 5):
                for m in range(4):
                    nc.tensor.matmul(gh[:, 64 * m:64 * m + 64],
                                     wt[k][:, _H0 + 128 * m:_H0 + 128 * m + 128],
                                     hk(k), start=False,
                                     stop=(m == 3 and k == 4))
            if not enc:
                for k in range(1, 5):
                    for m in range(4):
                        nc.tensor.matmul(gn[:, 64 * m:64 * m + 64],
                                         wt[k][:, _N0 + 128 * m:_N0 + 128 * m + 128],
                                         hk(k), start=False,
                                         stop=(m == 3 and k == 4))
            # phase D: y over h k-tiles
            if want_y:
                for k in range(1, 5):
                    nc.tensor.matmul(gy[:, :], wt[k][:, _Y0:_Y0 + 64],
                                     hk(k), start=False, stop=(k == 4))

        def emit_gates(grz, gn, gh, gy, ytile=None, yslot=0):
            """Gate math, split in column halves to shorten the serial chain:
            an = i_n + (1+tr)*hn2; h' = (1-z)*n + z*h. bf16 for SBUF-side
            tensors (2x/4x DVE modes); u2/an f32 (PSUM sources are f32)."""
            tr_t = gp.tile([128, 256], bf16, tag="tr_t")
            tz_t = gp.tile([128, 256], bf16, tag="tz_t")
            zz2 = gp.tile([128, 256], bf16, tag="zz2")
            nz2 = gp.tile([128, 256], bf16, tag="nz2")
            q = gp.tile([128, 256], bf16, tag="q")
            u2 = gp.tile([128, 256], f32, tag="u2")
            an = gp.tile([128, 256], f32, tag="an")
            tn = gp.tile([128, 256], bf16, tag="tn")
            mm1 = gp.tile([128, 256], bf16, tag="mm1")
            H0, H1 = slice(0, 128), slice(128, 256)

            # ACT: tanh only (no table switches)
            nc.scalar.activation(tr_t[:], grz[:, 0:256], TANH)
            nc.scalar.activation(tz_t[:], grz[:, 256:512], TANH)
            # prep ops (DVE, before the chain; only need tz_t)
            nc.vector.tensor_scalar(zz2[:], tz_t[:], 0.5, 0.5, MUL, ADD)
            nc.vector.tensor_scalar(nz2[:], tz_t[:], -0.5, 0.5, MUL, ADD)
            nc.gpsimd.tensor_tensor(q[:], zz2[:], hbf[:], MUL)    # z*h (bf16)
            # chain, halved: u2 -> an -> (tn on ACT) -> mm1 -> hbf
            for hh in (H0, H1):
                nc.vector.scalar_tensor_tensor(
                    u2[:, hh], tr_t[:, hh], 1.0, gh[:, hh], ADD, MUL)
                nc.vector.tensor_tensor(an[:, hh], u2[:, hh], gn[:, hh], ADD)
            nc.scalar.activation(tn[:, H0], an[:, H0], TANH)
            nc.scalar.activation(tn[:, H1], an[:, H1], TANH)
            for hh in (H0, H1):
                nc.vector.tensor_tensor(mm1[:, hh], nz2[:, hh], tn[:, hh], MUL)
                nc.vector.tensor_tensor(hbf[:, hh], mm1[:, hh], q[:, hh], ADD)
            if ytile is not None:
                nc.vector.tensor_copy(
                    ytile[:, yslot * BC:(yslot + 1) * BC], gy[:, :])
            nc.vector.tensor_tensor(hf32[:], mm1[:], q[:], ADD)   # h' f32

        def alloc_psum():
            grz = psum.tile([128, 512], f32, tag="grz")
            gn = psum.tile([128, 256], f32, tag="gn")
            gh = psum.tile([128, 256], f32, tag="gh")
            gy = psum.tile([64, 64], f32, tag="gy")
            return grz, gn, gh, gy

        def enc_step(rhs0):
            grz, gn, gh, gy = alloc_psum()
            emit_mms(grz, gn, gh, gy, rhs0, wte, enc=True, want_y=False)
            emit_gates(grz, gn, gh, gy)

        def dec_step(ytile, yslot):
            grz, gn, gh, gy = alloc_psum()
            emit_mms(grz, gn, gh, gy, rhs0d[0:65, :], wtd, enc=False, want_y=True)
            emit_gates(grz, gn, gh, gy, ytile=ytile, yslot=yslot)

        PE = mybir.EngineType.PE
        DVE = mybir.EngineType.DVE

        # ---- encoder ----
        if use_loops:
            with tc.For_i(0, s_steps * BC, ue * BC, hint_engines=(PE, DVE)) as iv:
                xch = xpool.tile([65, ue * BC], bf16, tag="xch")
                nc.sync.dma_start(xch[:], x_ext[:, bass.ds(iv, ue * BC)])
                for j in range(ue):
                    enc_step(xch[:, j * BC:(j + 1) * BC])
        else:
            for i0 in range(0, s_steps, ue):
                xch = xpool.tile([65, ue * BC], bf16, tag="xch")
                nc.sync.dma_start(xch[:], x_ext[:, i0 * BC:(i0 + ue) * BC])
                for j in range(ue):
                    enc_step(xch[:, j * BC:(j + 1) * BC])

        # ---- decoder (no bridge needed: Wl folded, no y feedback) ----
        if use_loops:
            with tc.For_i(0, t_steps * BC, ud * BC, hint_engines=(PE, DVE)) as iv:
                yt = ypool.tile([64, ud * BC], f32, tag="yt")
                for j in range(ud):
                    dec_step(yt, j)
                nc.sync.dma_start(y_ext[:, bass.ds(iv, ud * BC)], yt[:])
        else:
            for d0 in range(0, t_steps, ud):
                yt = ypool.tile([64, ud * BC], f32, tag="yt")
                for j in range(ud):
                    dec_step(yt, j)
                nc.sync.dma_start(y_ext[:, d0 * BC:(d0 + ud) * BC], yt[:])

        # ---- tail: y for the final hidden state -> slot T ----
        gy_t = psum.tile([64, 64], f32, tag="gy")
        nc.tensor.matmul(gy_t[:, :], wtd[0][0:65, _Y0:_Y0 + 64],
                         rhs0d[0:65, :], start=True, stop=False)
        for k in range(1, 5):
            nc.tensor.matmul(gy_t[:, :], wtd[k][:, _Y0:_Y0 + 64],
                             hbf[:, (k - 1) * 64:k * 64], start=False, stop=(k == 4))
        ylast = ypool.tile([64, BC], f32, tag="ylast")
        nc.vector.tensor_copy(ylast[:], gy_t[:, :])
        nc.sync.dma_start(y_ext[:, t_steps * BC:(t_steps + 1) * BC], ylast[:])

    nc.compile()
    return nc


def run(nc, w_bf, x_cores, trace=False):
    """Execute on 8 cores; returns per-core y arrays and BassKernelResults."""
    from concourse.bass_utils import run_bass_kernel_spmd
    in_maps = [{"w": w_bf, "xh": x_cores[c]} for c in range(NCORES)]
    res = run_bass_kernel_spmd(nc, in_maps, core_ids=list(range(NCORES)),
                               trace=trace)
    return [res.results[c]["y"] for c in range(NCORES)], res


_NC_CACHE = {}


def kernel(x, Wi, Wh, bi, bh, Wl, bl, targets=None, target_seq_len=T,
           teacher_forcing_rate=0, **_unused):
    x = np.asarray(x, np.float32)
    assert x.shape == (B, S, I), x.shape
    assert int(target_seq_len) == T
    w_bf = _build_weights(np.asarray(Wi, np.float32), np.asarray(Wh, np.float32),
                          np.asarray(bi, np.float32), np.asarray(bh, np.float32),
                          np.asarray(Wl, np.float32), np.asarray(bl, np.float32))
    x_cores = [_build_x(x[c * BC:(c + 1) * BC]) for c in range(NCORES)]

    key = (S, T)
    if key not in _NC_CACHE:
        _NC_CACHE[key] = build_program(S, T)
    ys, _ = run(_NC_CACHE[key], w_bf, x_cores)

    out = np.empty((B, T, I), np.float32)
    for c in range(NCORES):
        yc = ys[c].reshape(64, T + 1, BC)[:, 1:, :]   # [I, T, BC]
        out[c * BC:(c + 1) * BC] = yc.transpose(2, 1, 0)
    return out


if __name__ == "__main__":
    import reference
    inputs = reference.setup_inputs()
    out = kernel(**{k: np.asarray(v) if hasattr(v, "shape") else v
                    for k, v in inputs.items()})
    print("kernel out", out.shape, out.dtype)
